# revision 1
# baseline (speedup 1.0000x reference)
"""Bass/Trainium2 attention kernel for nn_AttentionModule_39462159515861.

Full inputs in, full output out. Sharding: 8 cores = (batch b in 0..3) x
(head-group g in 0..1), 8 heads per group. Each core computes QKV for its
heads, attention, and a partial output projection over its 512 inner dims;
the host sums the two partials per batch (tensor-parallel contraction).

Device-side layout choices (all transposes done on host, in numpy):
  xT     [1024, 2048]  x[b].T                  (c on partitions)
  wqkvT  [1024, 1536]  [wq_g*scale | wk_g | wv_g].T  (c on partitions)
  bqk    [1024]        q|k bias (q part pre-scaled)
  bv     [512]         v bias
  wpT    [512, 1024]   w_proj[:, g*512:(g+1)*512].T
  bph    [1024]        b_proj / 2  (each pair member adds half)
Output:
  part   [2048, 1024]  partial projection output
"""

import sys

sys.path.insert(0, "/opt/trn_rl_repo")

import numpy as np

import concourse.bass as bass
import concourse.mybir as mybir
from concourse import bacc
from concourse.tile import TileContext
from concourse.bass_utils import run_bass_kernel_spmd

DIM = 1024
HEADS = 16
HD = 64
B = 4
N = 2048
GH = 8           # heads per core
GI = GH * HD     # 512 inner dims per core
P = 128
FP = mybir.dt.float32
FPR = mybir.dt.float32r
SCALE = HD ** -0.5

USE_F32R = True  # float32r matmuls: full PE rate, ~tf32 precision


def _mm_cast(ap):
    return ap.bitcast(FPR) if USE_F32R else ap


def build_nc():
    nc = bacc.Bacc("TRN2", target_bir_lowering=False, debug=False, num_devices=8)

    xT = nc.dram_tensor("xT", [DIM, N], FP, kind="ExternalInput").ap()
    wqkvT = nc.dram_tensor("wqkvT", [DIM, 3 * GI], FP, kind="ExternalInput").ap()
    bqk = nc.dram_tensor("bqk", [2 * GI], FP, kind="ExternalInput").ap()
    bv = nc.dram_tensor("bv", [GI], FP, kind="ExternalInput").ap()
    wpT = nc.dram_tensor("wpT", [GI, DIM], FP, kind="ExternalInput").ap()
    bph = nc.dram_tensor("bph", [DIM], FP, kind="ExternalInput").ap()
    part = nc.dram_tensor("part", [N, DIM], FP, kind="ExternalOutput").ap()

    NC8 = DIM // P       # 8 c-chunks
    NT = N // P          # 16 token tiles
    N4 = N // 512        # 4 n-chunks of 512
    VW = HD + 1          # 65: v columns + ones column

    with TileContext(nc) as tc, nc.allow_low_precision(reason="fp32r matmul pipeline"):
        with (
            tc.tile_pool(name="persist", bufs=1) as persist,
            tc.tile_pool(name="small", bufs=1) as small,
        ):
            # Persistent SBUF tensors
            qk_sb = [persist.tile([P, N], FP, name=f"qk{i}") for i in range(8)]
            v_sb = [persist.tile([P, GH * VW], FP, name=f"v{i}") for i in range(NT)]
            cat_sb = [persist.tile([P, N], FP, name=f"cat{i}") for i in range(4)]

            bqk_sb = small.tile([P, 8], FP, name="bqk_sb")
            nc.sync.dma_start(out=bqk_sb, in_=bqk.rearrange("(jt p) -> p jt", p=P))
            bv_bc = small.tile([P, GI], FP, name="bv_bc")
            nc.sync.dma_start(
                out=bv_bc, in_=bv.rearrange("(one j) -> one j", one=1).partition_broadcast(P)
            )
            bp_bc = small.tile([P, DIM], FP, name="bp_bc")
            nc.sync.dma_start(
                out=bp_bc, in_=bph.rearrange("(one j) -> one j", one=1).partition_broadcast(P)
            )
            # ones columns of v_aug (memset f32, DVE-copy rounds to f32r)
            ones_f32 = small.tile([P, GH], FP, name="ones_f32")
            nc.vector.memset(ones_f32, 1.0)
            for mt in range(NT):
                vv = v_sb[mt].rearrange("p (h w) -> p h w", w=VW)
                nc.vector.tensor_copy(
                    _mm_cast(vv[:, :, HD : HD + 1]),
                    ones_f32.rearrange("p (h w) -> p h w", w=1),
                )
            ones_col = small.tile([1, HD], FP, name="ones_col")
            nc.vector.tensor_copy(_mm_cast(ones_col), ones_f32[0:1, 0:1].broadcast_to([1, HD]))

            # ---------------- Stage 1: QKV projection ----------------
            with (
                tc.tile_pool(name="wq_pool", bufs=1) as wq_pool,
                tc.tile_pool(name="x_pool", bufs=10) as x_pool,
                tc.tile_pool(name="ps1", bufs=6, space="PSUM") as ps1,
            ):
                wq_sb = [wq_pool.tile([P, 3 * GI], FP, name=f"wq{c}") for c in range(NC8)]
                for c in range(NC8):
                    nc.sync.dma_start(out=_mm_cast(wq_sb[c]), in_=_mm_cast(wqkvT[c * P : (c + 1) * P, :]))

                for n4 in range(N4):
                    nsl = slice(n4 * 512, (n4 + 1) * 512)
                    xs = []
                    for c in range(NC8):
                        xt = x_pool.tile([P, 512], FP, tag="xs")
                        nc.sync.dma_start(out=_mm_cast(xt), in_=_mm_cast(xT[c * P : (c + 1) * P, nsl]))
                        xs.append(xt)
                    # q,k: out [j 128, n 512] ; j-tiles 0..7 (q: 0-3, k: 4-7)
                    for jt in range(8):
                        ps = ps1.tile([P, 512], FP, tag="ps1t")
                        for c in range(NC8):
                            nc.tensor.matmul(
                                ps,
                                lhsT=_mm_cast(wq_sb[c][:, jt * P : (jt + 1) * P]),
                                rhs=_mm_cast(xs[c]),
                                start=(c == 0),
                                stop=(c == NC8 - 1),
                            )
                        nc.vector.tensor_scalar_add(
                            _mm_cast(qk_sb[jt][:, nsl]), ps, bqk_sb[:, jt : jt + 1]
                        )
                    # v: out [m 128, jv 512] ; 4 m-subtiles per n4
                    for ms in range(4):
                        mt = n4 * 4 + ms
                        ps = ps1.tile([P, 512], FP, tag="ps1t")
                        for c in range(NC8):
                            nc.tensor.matmul(
                                ps,
                                lhsT=_mm_cast(xs[c][:, ms * P : (ms + 1) * P]),
                                rhs=_mm_cast(wq_sb[c][:, 2 * GI : 3 * GI]),
                                start=(c == 0),
                                stop=(c == NC8 - 1),
                            )
                        vv = v_sb[mt].rearrange("p (h w) -> p h w", w=VW)
                        nc.vector.tensor_add(
                            _mm_cast(vv[:, :, 0:HD]),
                            ps.rearrange("p (h w) -> p h w", w=HD),
                            bv_bc.rearrange("p (h w) -> p h w", w=HD),
                        )

            # ---------------- Stage 2: attention ----------------
            with (
                tc.tile_pool(name="probs", bufs=6) as probs_pool,
                tc.tile_pool(name="zpool", bufs=4) as z_pool,
                tc.tile_pool(name="ps2", bufs=2, space="PSUM") as ps2,
                tc.tile_pool(name="pso", bufs=2, space="PSUM") as pso,
            ):
                for h in range(GH):
                    qt = h // 2
                    prow = (h % 2) * HD
                    qT_h = qk_sb[qt][prow : prow + HD, :]
                    kT_h = qk_sb[4 + qt][prow : prow + HD, :]
                    for n2 in range(2):
                        po = [
                            pso.tile([P, 512], FP, tag="po", name=f"po{h}_{n2}_{i}")
                            for i in range(2)
                        ]
                        for mt in range(NT):
                            ps = ps2.tile([P, 1024], FP, tag="ps_s")
                            for i in range(2):
                                nc.tensor.matmul(
                                    ps[:, i * 512 : (i + 1) * 512],
                                    lhsT=_mm_cast(kT_h[:, mt * P : (mt + 1) * P]),
                                    rhs=_mm_cast(
                                        qT_h[:, n2 * 1024 + i * 512 : n2 * 1024 + (i + 1) * 512]
                                    ),
                                    start=True,
                                    stop=True,
                                )
                            pt = probs_pool.tile([P, 1024], FP, tag="pt")
                            nc.scalar.activation(
                                _mm_cast(pt), ps, mybir.ActivationFunctionType.Exp
                            )
                            for i in range(2):
                                nc.tensor.matmul(
                                    po[i][0:VW, :],
                                    lhsT=_mm_cast(v_sb[mt][:, h * VW : (h + 1) * VW]),
                                    rhs=_mm_cast(pt[:, i * 512 : (i + 1) * 512]),
                                    start=(mt == 0),
                                    stop=(mt == NT - 1),
                                )
                        for i in range(2):
                            nsl = slice(n2 * 1024 + i * 512, n2 * 1024 + (i + 1) * 512)
                            zr = z_pool.tile([1, 512], FP, tag="zr")
                            nc.vector.reciprocal(_mm_cast(zr), po[i][HD : HD + 1, :])
                            zbp = ps2.tile([HD, 512], FP, tag="zb")
                            nc.tensor.matmul(
                                zbp,
                                lhsT=_mm_cast(ones_col),
                                rhs=_mm_cast(zr),
                                start=True,
                                stop=True,
                            )
                            zb = z_pool.tile([HD, 512], FP, tag="zb_sb")
                            nc.vector.tensor_copy(zb, zbp)
                            nc.vector.tensor_mul(
                                _mm_cast(cat_sb[qt][prow : prow + HD, nsl]), po[i][0:HD, :], zb
                            )

            # ---------------- Stage 3: output projection (partial) ----------------
            with (
                tc.tile_pool(name="wp_pool", bufs=1) as wp_pool,
                tc.tile_pool(name="outp", bufs=4) as outp,
                tc.tile_pool(name="ps3", bufs=4, space="PSUM") as ps3,
            ):
                wp_sb = [wp_pool.tile([P, DIM], FP, name=f"wp{i}") for i in range(4)]
                for i in range(4):
                    nc.sync.dma_start(out=_mm_cast(wp_sb[i]), in_=_mm_cast(wpT[i * P : (i + 1) * P, :]))
                for nt in range(NT):
                    for o2 in range(2):
                        osl = slice(o2 * 512, (o2 + 1) * 512)
                        ps = ps3.tile([P, 512], FP, tag="ps_p")
                        for ic in range(4):
                            nc.tensor.matmul(
                                ps,
                                lhsT=_mm_cast(cat_sb[ic][:, nt * P : (nt + 1) * P]),
                                rhs=_mm_cast(wp_sb[ic][:, osl]),
                                start=(ic == 0),
                                stop=(ic == 3),
                            )
                        ot = outp.tile([P, 512], FP, tag="ot")
                        nc.vector.tensor_add(ot, ps, bp_bc[:, osl])
                        nc.sync.dma_start(
                            out=part[nt * P : (nt + 1) * P, osl], in_=ot
                        )

    nc.compile()
    return nc


_NC = None


def _get_nc():
    global _NC
    if _NC is None:
        _NC = build_nc()
    return _NC


def _make_in_maps(x, w_qkv, b_qkv, w_proj, b_proj):
    x = np.asarray(x, np.float32)
    w_qkv = np.asarray(w_qkv, np.float32)
    b_qkv = np.asarray(b_qkv, np.float32)
    w_proj = np.asarray(w_proj, np.float32)
    b_proj = np.asarray(b_proj, np.float32)
    in_maps = []
    for c in range(8):
        b, g = c // 2, c % 2
        hsl = slice(g * GI, (g + 1) * GI)
        wq = w_qkv[0 * DIM + g * GI : 0 * DIM + (g + 1) * GI] * SCALE
        wk = w_qkv[1 * DIM + g * GI : 1 * DIM + (g + 1) * GI]
        wv = w_qkv[2 * DIM + g * GI : 2 * DIM + (g + 1) * GI]
        wqkvT = np.ascontiguousarray(np.concatenate([wq, wk, wv], 0).T)
        bq = b_qkv[0 * DIM + g * GI : 0 * DIM + (g + 1) * GI] * SCALE
        bk = b_qkv[1 * DIM + g * GI : 1 * DIM + (g + 1) * GI]
        bv_ = b_qkv[2 * DIM + g * GI : 2 * DIM + (g + 1) * GI]
        in_maps.append(
            {
                "xT": np.ascontiguousarray(x[b].T),
                "wqkvT": wqkvT,
                "bqk": np.ascontiguousarray(np.concatenate([bq, bk])),
                "bv": np.ascontiguousarray(bv_),
                "wpT": np.ascontiguousarray(w_proj[:, hsl].T),
                "bph": np.ascontiguousarray(b_proj * 0.5),
            }
        )
    return in_maps


def _run(in_maps, trace=False):
    nc = _get_nc()
    return run_bass_kernel_spmd(nc, in_maps, core_ids=list(range(8)), trace=trace)


def kernel(x, w_qkv, b_qkv, w_proj, b_proj):
    in_maps = _make_in_maps(x, w_qkv, b_qkv, w_proj, b_proj)
    res = _run(in_maps, trace=False)
    parts = [np.asarray(res.results[c]["part"]) for c in range(8)]
    out = np.empty((B, N, DIM), np.float32)
    for b in range(B):
        out[b] = parts[2 * b] + parts[2 * b + 1]
    return out


def bench(x, w_qkv, b_qkv, w_proj, b_proj, iters=16):
    """Returns (out, approx_exec_ns_per_iter, None). NTFF profiling is
    unavailable under this axon client; instead chain `iters` kernel
    executions inside one jit (serialized via a data dependency) with
    device-resident inputs, and report wall/iters. Slight overestimate:
    includes a per-iter output-buffer memset and one input add."""
    import time

    import jax
    import jax.numpy as jnp
    from jax.sharding import Mesh, PartitionSpec
    from jax.experimental.shard_map import shard_map
    from concourse import bass2jax

    nc = _get_nc()
    bass2jax.install_neuronx_cc_hook()
    in_maps = _make_in_maps(x, w_qkv, b_qkv, w_proj, b_proj)

    in_names, out_names, out_avals = [], [], []
    for alloc in nc.m.functions[0].allocations:
        if not isinstance(alloc, mybir.MemoryLocationSet):
            continue
        name = alloc.memorylocations[0].name
        if alloc.kind == "ExternalInput":
            if nc.partition_id_tensor and name == nc.partition_id_tensor.name:
                continue
            in_names.append(name)
        elif alloc.kind == "ExternalOutput":
            out_names.append(name)
            out_avals.append(
                jax.core.ShapedArray(tuple(alloc.tensor_shape), mybir.dt.np(alloc.dtype))
            )
    n_params = len(in_names)
    partition_name = nc.partition_id_tensor.name if nc.partition_id_tensor else None
    all_in_names = tuple(in_names) + tuple(out_names)
    if partition_name is not None:
        all_in_names = all_in_names + (partition_name,)

    def _exec(*args):
        operands = list(args)
        if partition_name is not None:
            operands.append(bass2jax.partition_id_tensor())
        outs = bass2jax._bass_exec_p.bind(
            *operands,
            out_avals=tuple(out_avals),
            in_names=all_in_names,
            out_names=tuple(out_names),
            lowering_input_output_aliases=(),
            sim_require_finite=True,
            sim_require_nnan=True,
            nc=nc,
        )
        return tuple(outs)

    def _chained(*args):
        return _exec(*args)

    mesh = Mesh(np.asarray(jax.devices()[:8]), ("core",))
    sharded = jax.jit(
        shard_map(
            _chained,
            mesh=mesh,
            in_specs=(PartitionSpec("core"),) * (n_params + len(out_names)),
            out_specs=(PartitionSpec("core"),) * len(out_names),
            check_rep=False,
        )
    )
    per_core = [[np.asarray(m[n]) for n in in_names] for m in in_maps]
    concat_in = [
        np.concatenate([per_core[c][i] for c in range(8)], 0) for i in range(n_params)
    ]
    concat_in += [
        np.zeros((8 * av.shape[0], *av.shape[1:]), av.dtype) for av in out_avals
    ]
    dev_in = [jax.device_put(a) for a in concat_in]
    outs = sharded(*dev_in)
    jax.block_until_ready(outs)  # compile + warm
    best = None
    for _ in range(max(iters, 3)):
        t0 = time.perf_counter()
        outs = sharded(*dev_in)
        jax.block_until_ready(outs)
        dt = time.perf_counter() - t0
        best = dt if best is None else min(best, dt)
    parts_cat = np.asarray(outs[0]).reshape(8, N, DIM)
    out = np.empty((B, N, DIM), np.float32)
    for b in range(B):
        out[b] = parts_cat[2 * b] + parts_cat[2 * b + 1]
    return out, int(best * 1e9), None

